# revision 4
# baseline (speedup 1.0000x reference)
"""Multi-head attention (B=1, S=4096, D=1024, H=16, causal) on 8 Trainium2
NeuronCores.

Sharding: tensor-parallel over heads — each core owns 2 heads (128 of the
1024 projection dims). Wq/Wk/Wv are split column-wise, Wo row-wise; each
core computes a full [S, D] partial of the output projection and the
all-reduce is done on the host by summing the 8 partials (+ Wo_b once).

Per-core device kernel (all matmuls in f32r at N=512 → full PE rate):
  qT/kT/vT projections produce [c=128, S] layouts directly (contract dim D
  streams from host-pretransposed Q^T/K^T/V^T in HBM); v is PE-transposed
  per 128-block into an augmented [k, 65] layout (ones column ⇒ softmax
  denominator falls out of the attn@V matmul as PSUM row 64).
  Scores are computed transposed (scoresT[k, q] = k q^T) so softmax exp is
  the PSUM eviction (ACT, scale=1/8, additive -1e9 causal mask on the 4
  diagonal blocks only, fully-masked blocks skipped) and attn@V needs no
  transposes. Normalization (1/denom) is broadcast across partitions with a
  K=1 ones matmul, applied by DVE, and the normalized [c, q] tiles are the
  stationary operands of the final Wo matmul.
"""

import numpy as np

D = 1024
H = 16
DK = D // H  # 64
S = 4096
NCORES = 8
CD = 128          # c-dims (2 heads) per core
ST = 512          # s/q tile
NST = S // ST     # 8
KB = 128          # k block
NKB = S // KB     # 32
SLOT = 2 * (DK + 1)  # 130: v_sb cols per k-block (2 heads x (64 dims + ones))

_compiled = [None]


def _build():
    import concourse.bacc as bacc
    import concourse.mybir as mybir
    import concourse.tile as tile

    f32 = mybir.dt.float32
    f32r = mybir.dt.float32r
    EXP = mybir.ActivationFunctionType.Exp
    ADD = mybir.AluOpType.add
    MULT = mybir.AluOpType.mult

    nc = bacc.Bacc(None, target_bir_lowering=False)

    QT = nc.dram_tensor("qt", [D, S], f32r, kind="ExternalInput")
    KT = nc.dram_tensor("kt", [D, S], f32r, kind="ExternalInput")
    VT = nc.dram_tensor("vt", [D, S], f32r, kind="ExternalInput")
    WQ = nc.dram_tensor("wq", [D, CD], f32r, kind="ExternalInput")
    WK = nc.dram_tensor("wk", [D, CD], f32r, kind="ExternalInput")
    WV = nc.dram_tensor("wv", [D, CD], f32r, kind="ExternalInput")
    BQ = nc.dram_tensor("bq", [1, CD], f32r, kind="ExternalInput")
    BK = nc.dram_tensor("bk", [1, CD], f32r, kind="ExternalInput")
    BV = nc.dram_tensor("bv", [1, CD], f32r, kind="ExternalInput")
    WO0 = nc.dram_tensor("wo0", [DK, D], f32r, kind="ExternalInput")
    WO1 = nc.dram_tensor("wo1", [DK, D], f32r, kind="ExternalInput")
    MSK = nc.dram_tensor("msk", [KB, 4 * ST], f32, kind="ExternalInput")
    ONEV = nc.dram_tensor("onev", [KB, NKB, 1], f32r, kind="ExternalInput")
    ONES = nc.dram_tensor("ones", [1, ST], f32r, kind="ExternalInput")
    EYE = nc.dram_tensor("eye", [128, 128], f32, kind="ExternalInput")
    OUT = nc.dram_tensor("out", [S, D], f32, kind="ExternalOutput")

    def r(ap):
        return ap.bitcast(f32r)

    with tile.TileContext(nc) as tc:
        with (
            tc.tile_pool(name="const", bufs=1) as const,
            tc.tile_pool(name="qin", bufs=2) as qin_p,
            tc.tile_pool(name="kin", bufs=2) as kin_p,
            tc.tile_pool(name="vin", bufs=2) as vin_p,
            tc.tile_pool(name="vtx", bufs=2) as vtx_p,
            tc.tile_pool(name="expp", bufs=3) as exp_p,
            tc.tile_pool(name="osb", bufs=2) as osb_p,
            tc.tile_pool(name="rsb", bufs=2) as rsb_p,
            tc.tile_pool(name="oout", bufs=3) as oout_p,
            tc.tile_pool(name="psA", bufs=2, space="PSUM") as psA,
            tc.tile_pool(name="psS", bufs=2, space="PSUM") as psS,
            tc.tile_pool(name="psO", bufs=2, space="PSUM") as psO,
        ):
            # ---- static SBUF tensors ----
            qT_sb = const.tile([CD, S], f32r, tag="qT")
            kT_sb = const.tile([CD, S], f32r, tag="kT")
            v_sb = const.tile([128, NKB, SLOT], f32r, tag="vsb")
            woL0 = const.tile([DK, S], f32r, tag="woL0")
            woL1 = const.tile([DK, S], f32r, tag="woL1")
            wq_sb = const.tile([128, D], f32r, tag="wq")
            wk_sb = const.tile([128, D], f32r, tag="wk")
            wv_sb = const.tile([128, D], f32r, tag="wv")
            woR0 = const.tile([DK, D], f32r, tag="woR0")
            woR1 = const.tile([DK, D], f32r, tag="woR1")
            mask_sb = const.tile([KB, 4 * ST], f32, tag="mask")
            eye_sb = const.tile([128, 128], f32, tag="eye")
            bq_sb = const.tile([1, CD], f32r, tag="bq")
            bk_sb = const.tile([1, CD], f32r, tag="bk")
            bv_sb = const.tile([1, CD], f32r, tag="bv")
            ones_s = const.tile([1, ST], f32r, tag="ones_s")
            onesP = const.tile([65, DK], f32, tag="onesP")

            # ---- const loads ----
            # weight lhsT tiles: wq_sb[:, d*128:(d+1)*128] = WQ[d-block rows]
            for w_sb, W in ((wq_sb, WQ), (wk_sb, WK), (wv_sb, WV)):
                nc.sync.dma_start(
                    out=w_sb.rearrange("p (t c) -> p t c", c=CD),
                    in_=W.rearrange("(t p) c -> p t c", p=128),
                )
            nc.sync.dma_start(out=woR0[:], in_=WO0[:])
            nc.sync.dma_start(out=woR1[:], in_=WO1[:])
            nc.sync.dma_start(out=mask_sb[:], in_=MSK[:])
            nc.sync.dma_start(out=eye_sb[:], in_=EYE[:])
            nc.sync.dma_start(out=bq_sb[:], in_=BQ[:])
            nc.sync.dma_start(out=bk_sb[:], in_=BK[:])
            nc.sync.dma_start(out=bv_sb[:], in_=BV[:])
            nc.sync.dma_start(out=ones_s[:], in_=ONES[:])
            nc.vector.memset(onesP[64:65, :], 1.0)
            # ones columns of v_aug slots
            nc.sync.dma_start(out=v_sb[:, :, DK : DK + 1], in_=ONEV[:])
            nc.sync.dma_start(out=v_sb[:, :, SLOT - 1 : SLOT], in_=ONEV[:])

            QTr = QT.rearrange("(g t p) s -> g p t s", g=2, p=128)
            KTr = KT.rearrange("(g t p) s -> g p t s", g=2, p=128)
            VTr = VT.rearrange("(g t p) s -> g p t s", g=2, p=128)

            def project(st, src_r, w_sb, b_sb, in_pool, dst_ap):
                """dst_ap [128, ST] = (W X + b)^T tile for s-range st."""
                ps = psA.tile([128, ST], f32, tag="pp", name=f"pp{st}")
                for g in range(2):
                    xin = in_pool.tile([128, 4, ST], f32r, tag="xin",
                                       name=f"xin{st}_{g}")
                    nc.sync.dma_start(
                        out=xin[:],
                        in_=src_r[g][:, :, st * ST : (st + 1) * ST],
                    )
                    for t in range(4):
                        d = 4 * g + t
                        nc.tensor.matmul(
                            ps[:],
                            lhsT=(w_sb[:, d * CD : (d + 1) * CD]),
                            rhs=(xin[:, t, :]),
                            start=(d == 0),
                            stop=False,
                        )
                # bias: += b ⊗ ones_s  (K=1)
                nc.tensor.matmul(
                    ps[:], lhsT=b_sb[:], rhs=ones_s[:],
                    start=False, stop=True,
                )
                nc.vector.tensor_copy(dst_ap, ps[:])
                return ps

            def attn(qt, h):
                nkb = 4 * qt + 4
                po = psO.tile([65, ST], f32, tag="po", name=f"po{qt}_{h}")
                for pr in range(nkb // 2):
                    ps = psS.tile([128, 2 * ST], f32, tag="ps",
                                  name=f"ps{qt}_{h}_{pr}")
                    for j in range(2):
                        kb = 2 * pr + j
                        nc.tensor.matmul(
                            ps[:, j * ST : (j + 1) * ST],
                            lhsT=(kT_sb[64 * h : 64 * h + 64,
                                         kb * KB : (kb + 1) * KB]),
                            rhs=(qT_sb[64 * h : 64 * h + 64,
                                        qt * ST : (qt + 1) * ST]),
                            start=True,
                            stop=True,
                        )
                        rel = kb - 4 * qt
                        if rel >= 0:  # diagonal block: additive causal mask
                            nc.vector.tensor_tensor(
                                out=ps[:, j * ST : (j + 1) * ST],
                                in0=ps[:, j * ST : (j + 1) * ST],
                                in1=mask_sb[:, rel * ST : (rel + 1) * ST],
                                op=ADD,
                            )
                    ex = exp_p.tile([128, 2 * ST], f32r, tag="ex",
                                    name=f"ex{qt}_{h}_{pr}")
                    nc.scalar.activation(ex[:], ps[:], EXP, scale=0.125)
                    for j in range(2):
                        kb = 2 * pr + j
                        nc.tensor.matmul(
                            po[:],
                            lhsT=(v_sb[:, kb, h * 65 : h * 65 + 65]),
                            rhs=(ex[:, j * ST : (j + 1) * ST]),
                            start=(pr == 0 and j == 0),
                            stop=(pr == nkb // 2 - 1 and j == 1),
                        )
                o_sb = osb_p.tile([65, ST], f32, tag="o", name=f"o{qt}_{h}")
                nc.vector.tensor_copy(o_sb[:], po[:])
                # denom (row 64) broadcast to 64 partitions via K=1 matmul
                pb = psO.tile([DK, ST], f32, tag="po", name=f"pb{qt}_{h}")
                nc.tensor.matmul(
                    pb[:], lhsT=onesP[64:65, :], rhs=o_sb[64:65, :],
                    start=True, stop=True,
                )
                r_sb = rsb_p.tile([DK, ST], f32, tag="r", name=f"r{qt}_{h}")
                nc.vector.reciprocal(r_sb[:], pb[:])
                woL = woL0 if h == 0 else woL1
                nc.vector.tensor_tensor(
                    out=woL[:, qt * ST : (qt + 1) * ST],
                    in0=o_sb[0:64, :],
                    in1=r_sb[:],
                    op=MULT,
                )

            def wo(qt):
                for qb in range(4):
                    q0 = qt * ST + qb * 128
                    for nt in range(2):
                        pw = psA.tile([128, ST], f32, tag="pp",
                                      name=f"pw{qt}_{qb}_{nt}")
                        nc.tensor.matmul(
                            pw[:],
                            lhsT=(woL0[:, q0 : q0 + 128]),
                            rhs=(woR0[:, nt * ST : (nt + 1) * ST]),
                            start=True, stop=False,
                        )
                        nc.tensor.matmul(
                            pw[:],
                            lhsT=(woL1[:, q0 : q0 + 128]),
                            rhs=(woR1[:, nt * ST : (nt + 1) * ST]),
                            start=False, stop=True,
                        )
                        ob = oout_p.tile([128, ST], f32, tag="ob",
                                         name=f"ob{qt}_{qb}_{nt}")
                        nc.vector.tensor_copy(ob[:], pw[:])
                        nc.sync.dma_start(
                            out=OUT[q0 : q0 + 128, nt * ST : (nt + 1) * ST],
                            in_=ob[:],
                        )

            for st in range(NST):
                project(st, QTr, wq_sb, bq_sb, qin_p,
                        qT_sb[:, st * ST : (st + 1) * ST])
                project(st, KTr, wk_sb, bk_sb, kin_p,
                        kT_sb[:, st * ST : (st + 1) * ST])
                vtx = vtx_p.tile([128, ST], f32, tag="vtx", name=f"vtx{st}")
                project(st, VTr, wv_sb, bv_sb, vin_p, vtx[:])
                # transpose vT [c, s] blocks into v_sb [s, c] aug slots
                for qb in range(4):
                    kb = 4 * st + qb
                    tp = psA.tile([128, 128], f32, tag="pp", name=f"pt{kb}")
                    nc.tensor.transpose(
                        tp[:], vtx[:, qb * 128 : (qb + 1) * 128], eye_sb[:]
                    )
                    nc.vector.tensor_copy(v_sb[:, kb, 0:DK], tp[:, 0:DK])
                    nc.vector.tensor_copy(
                        v_sb[:, kb, DK + 1 : SLOT - 1], tp[:, DK:CD]
                    )
                attn(st, 0)
                attn(st, 1)
                wo(st)

    nc.compile()
    return nc


def _prep_inputs(Q, K, V, Wq_w, Wq_b, Wk_w, Wk_b, Wv_w, Wv_b, Wo_w, Wo_b):
    f = np.float32
    QT = np.ascontiguousarray(Q[0].T, dtype=f)
    KT = np.ascontiguousarray(K[0].T, dtype=f)
    VT = np.ascontiguousarray(V[0].T, dtype=f)
    # diagonal-block additive causal masks: m[p, r*ST + f] = 0 if 128r+p<=f
    p = np.arange(KB)[:, None]
    fidx = np.arange(ST)[None, :]
    msk = np.concatenate(
        [np.where(128 * rel + p <= fidx, 0.0, -1e9) for rel in range(4)],
        axis=1,
    ).astype(f)
    eye = np.eye(128, dtype=f)
    WoT = np.ascontiguousarray(Wo_w.T, dtype=f)  # [in, out]

    in_maps = []
    for c in range(NCORES):
        c0 = CD * c
        in_maps.append({
            "qt": QT, "kt": KT, "vt": VT,
            "wq": np.ascontiguousarray(Wq_w[c0 : c0 + CD, :].T, dtype=f),
            "wk": np.ascontiguousarray(Wk_w[c0 : c0 + CD, :].T, dtype=f),
            "wv": np.ascontiguousarray(Wv_w[c0 : c0 + CD, :].T, dtype=f),
            "bq": np.ascontiguousarray(Wq_b[None, c0 : c0 + CD], dtype=f),
            "bk": np.ascontiguousarray(Wk_b[None, c0 : c0 + CD], dtype=f),
            "bv": np.ascontiguousarray(Wv_b[None, c0 : c0 + CD], dtype=f),
            "wo0": np.ascontiguousarray(WoT[c0 : c0 + DK, :], dtype=f),
            "wo1": np.ascontiguousarray(WoT[c0 + DK : c0 + CD, :], dtype=f),
            "msk": msk, "eye": eye,
            "onev": np.ones((KB, NKB, 1), f), "ones": np.ones((1, ST), f),
        })
    return in_maps


def _numpy_fallback(Q, K, V, Wq_w, Wq_b, Wk_w, Wk_b, Wv_w, Wv_b, Wo_w, Wo_b,
                    mask):
    q = (Q @ Wq_w.T + Wq_b).reshape(1, S, H, DK).transpose(0, 2, 1, 3)
    k = (K @ Wk_w.T + Wk_b).reshape(1, S, H, DK).transpose(0, 2, 1, 3)
    v = (V @ Wv_w.T + Wv_b).reshape(1, S, H, DK).transpose(0, 2, 1, 3)
    scores = np.einsum("bhqd,bhkd->bhqk", q, k) / np.sqrt(DK).astype(np.float32)
    scores = np.where(mask == 0, np.float32(-1e9), scores)
    scores -= scores.max(axis=-1, keepdims=True)
    e = np.exp(scores)
    attn = e / e.sum(axis=-1, keepdims=True)
    out = np.einsum("bhqk,bhkd->bhqd", attn, v)
    out = out.transpose(0, 2, 1, 3).reshape(1, S, D)
    return (out @ Wo_w.T + Wo_b).astype(np.float32)


def kernel(Q, K, V, Wq_w, Wq_b, Wk_w, Wk_b, Wv_w, Wv_b, Wo_w, Wo_b, mask,
           **run_kwargs):
    Q = np.asarray(Q); K = np.asarray(K); V = np.asarray(V)
    Wq_w = np.asarray(Wq_w); Wq_b = np.asarray(Wq_b)
    Wk_w = np.asarray(Wk_w); Wk_b = np.asarray(Wk_b)
    Wv_w = np.asarray(Wv_w); Wv_b = np.asarray(Wv_b)
    Wo_w = np.asarray(Wo_w); Wo_b = np.asarray(Wo_b)
    mask = np.asarray(mask)

    causal = np.array_equal(
        mask.reshape(S, S), np.tril(np.ones((S, S), mask.dtype))
    )
    if not causal:
        return _numpy_fallback(Q, K, V, Wq_w, Wq_b, Wk_w, Wk_b, Wv_w, Wv_b,
                               Wo_w, Wo_b, mask)

    from concourse.bass_utils import run_bass_kernel_spmd

    if _compiled[0] is None:
        _compiled[0] = _build()
    nc = _compiled[0]

    in_maps = _prep_inputs(Q, K, V, Wq_w, Wq_b, Wk_w, Wk_b, Wv_w, Wv_b,
                           Wo_w, Wo_b)
    res = run_bass_kernel_spmd(nc, in_maps, list(range(NCORES)), **run_kwargs)
    out = np.zeros((S, D), np.float32)
    for cres in res.results:
        out += cres["out"]
    out += Wo_b.astype(np.float32)
    if run_kwargs:
        kernel.last_result = res
    return out.reshape(1, S, D)


# revision 5
# speedup vs baseline: 1.1478x; 1.1478x over previous
"""Multi-head attention (B=1, S=4096, D=1024, H=16, causal) on 8 Trainium2
NeuronCores.

Sharding: tensor-parallel over heads — each core owns 2 heads (128 of the
1024 projection dims). Wq/Wk/Wv are split column-wise, Wo row-wise; each
core computes a full [S, D] partial of the output projection and the
all-reduce is done on the host by summing the 8 partials (+ Wo_b once).

Per-core device kernel (all matmuls in f32r at N=512 → full PE rate):
  qT/kT/vT projections produce [c=128, S] layouts directly (contract dim D
  streams from host-pretransposed Q^T/K^T/V^T in HBM); v is PE-transposed
  per 128-block into an augmented [k, 65] layout (ones column ⇒ softmax
  denominator falls out of the attn@V matmul as PSUM row 64).
  Scores are computed transposed (scoresT[k, q] = k q^T) so softmax exp is
  the PSUM eviction (ACT, scale=1/8, additive -1e9 causal mask on the 4
  diagonal blocks only, fully-masked blocks skipped) and attn@V needs no
  transposes. Normalization (1/denom) is broadcast across partitions with a
  K=1 ones matmul, applied by DVE, and the normalized [c, q] tiles are the
  stationary operands of the final Wo matmul.
"""

import numpy as np

D = 1024
H = 16
DK = D // H  # 64
S = 4096
NCORES = 8
CD = 128          # c-dims (2 heads) per core
ST = 512          # s/q tile
NST = S // ST     # 8
KB = 128          # k block
NKB = S // KB     # 32
SLOT = 2 * (DK + 1)  # 130: v_sb cols per k-block (2 heads x (64 dims + ones))

_compiled = [None]


def _build():
    import concourse.bacc as bacc
    import concourse.mybir as mybir
    import concourse.tile as tile

    f32 = mybir.dt.float32
    f32r = mybir.dt.float32r
    EXP = mybir.ActivationFunctionType.Exp
    ADD = mybir.AluOpType.add
    MULT = mybir.AluOpType.mult

    nc = bacc.Bacc(None, target_bir_lowering=False)

    QT = nc.dram_tensor("qt", [D, S], f32r, kind="ExternalInput")
    KT = nc.dram_tensor("kt", [D, S], f32r, kind="ExternalInput")
    VT = nc.dram_tensor("vt", [D, S], f32r, kind="ExternalInput")
    WQ = nc.dram_tensor("wq", [D, CD], f32r, kind="ExternalInput")
    WK = nc.dram_tensor("wk", [D, CD], f32r, kind="ExternalInput")
    WV = nc.dram_tensor("wv", [D, CD], f32r, kind="ExternalInput")
    BQ = nc.dram_tensor("bq", [1, CD], f32r, kind="ExternalInput")
    BK = nc.dram_tensor("bk", [1, CD], f32r, kind="ExternalInput")
    BV = nc.dram_tensor("bv", [1, CD], f32r, kind="ExternalInput")
    WO0 = nc.dram_tensor("wo0", [DK, D], f32r, kind="ExternalInput")
    WO1 = nc.dram_tensor("wo1", [DK, D], f32r, kind="ExternalInput")
    MSK = nc.dram_tensor("msk", [KB, 4 * ST], f32, kind="ExternalInput")
    ONEV = nc.dram_tensor("onev", [KB, NKB, 1], f32r, kind="ExternalInput")
    ONES = nc.dram_tensor("ones", [1, ST], f32r, kind="ExternalInput")
    ONEP = nc.dram_tensor("onep", [1, DK], f32r, kind="ExternalInput")
    EYE = nc.dram_tensor("eye", [128, 128], f32, kind="ExternalInput")
    OUT = nc.dram_tensor("out", [S, D], f32, kind="ExternalOutput")

    def r(ap):
        return ap.bitcast(f32r)

    with tile.TileContext(nc) as tc:
        with (
            tc.tile_pool(name="const", bufs=1) as const,
            tc.tile_pool(name="qin", bufs=2) as qin_p,
            tc.tile_pool(name="kin", bufs=2) as kin_p,
            tc.tile_pool(name="vin", bufs=2) as vin_p,
            tc.tile_pool(name="vtx", bufs=2) as vtx_p,
            tc.tile_pool(name="expp", bufs=3) as exp_p,
            tc.tile_pool(name="osb", bufs=2) as osb_p,
            tc.tile_pool(name="rsb", bufs=2) as rsb_p,
            tc.tile_pool(name="oout", bufs=3) as oout_p,
            tc.tile_pool(name="psA", bufs=2, space="PSUM") as psA,
            tc.tile_pool(name="psS", bufs=2, space="PSUM") as psS,
            tc.tile_pool(name="psO", bufs=2, space="PSUM") as psO,
        ):
            # ---- static SBUF tensors ----
            qT_sb = const.tile([CD, S], f32r, tag="qT")
            kT_sb = const.tile([CD, S], f32r, tag="kT")
            v_sb = const.tile([128, NKB, SLOT], f32r, tag="vsb")
            woL0 = const.tile([DK, S], f32r, tag="woL0")
            woL1 = const.tile([DK, S], f32r, tag="woL1")
            wq_sb = const.tile([128, D], f32r, tag="wq")
            wk_sb = const.tile([128, D], f32r, tag="wk")
            wv_sb = const.tile([128, D], f32r, tag="wv")
            woR0 = const.tile([DK, D], f32r, tag="woR0")
            woR1 = const.tile([DK, D], f32r, tag="woR1")
            mask_sb = const.tile([KB, 4 * ST], f32, tag="mask")
            eye_sb = const.tile([128, 128], f32, tag="eye")
            bq_sb = const.tile([1, CD], f32r, tag="bq")
            bk_sb = const.tile([1, CD], f32r, tag="bk")
            bv_sb = const.tile([1, CD], f32r, tag="bv")
            ones_s = const.tile([1, ST], f32r, tag="ones_s")
            onesP = const.tile([65, DK], f32r, tag="onesP")

            # ---- const loads ----
            # weight lhsT tiles: wq_sb[:, d*128:(d+1)*128] = WQ[d-block rows]
            for w_sb, W in ((wq_sb, WQ), (wk_sb, WK), (wv_sb, WV)):
                nc.sync.dma_start(
                    out=w_sb.rearrange("p (t c) -> p t c", c=CD),
                    in_=W.rearrange("(t p) c -> p t c", p=128),
                )
            nc.sync.dma_start(out=bq_sb[:], in_=BQ[:])
            nc.sync.dma_start(out=bk_sb[:], in_=BK[:])
            nc.sync.dma_start(out=bv_sb[:], in_=BV[:])
            nc.sync.dma_start(out=ones_s[:], in_=ONES[:])
            nc.sync.dma_start(out=eye_sb[:], in_=EYE[:])
            nc.sync.dma_start(out=onesP[64:65, :], in_=ONEP[:])
            # ones columns of v_aug slots
            nc.sync.dma_start(out=v_sb[:, :, DK : DK + 1], in_=ONEV[:])
            nc.sync.dma_start(out=v_sb[:, :, SLOT - 1 : SLOT], in_=ONEV[:])
            nc.sync.dma_start(out=mask_sb[:], in_=MSK[:])
            nc.sync.dma_start(out=woR0[:], in_=WO0[:])
            nc.sync.dma_start(out=woR1[:], in_=WO1[:])

            QTr = QT.rearrange("(g t p) s -> g p t s", g=2, p=128)
            KTr = KT.rearrange("(g t p) s -> g p t s", g=2, p=128)
            VTr = VT.rearrange("(g t p) s -> g p t s", g=2, p=128)

            def project(st, src_r, w_sb, b_sb, in_pool, dst_ap):
                """dst_ap [128, ST] = (W X + b)^T tile for s-range st."""
                ps = psA.tile([128, ST], f32, tag="pp", name=f"pp{st}")
                for g in range(2):
                    xin = in_pool.tile([128, 4, ST], f32r, tag="xin",
                                       name=f"xin{st}_{g}")
                    nc.sync.dma_start(
                        out=xin[:],
                        in_=src_r[g][:, :, st * ST : (st + 1) * ST],
                    )
                    for t in range(4):
                        d = 4 * g + t
                        nc.tensor.matmul(
                            ps[:],
                            lhsT=(w_sb[:, d * CD : (d + 1) * CD]),
                            rhs=(xin[:, t, :]),
                            start=(d == 0),
                            stop=False,
                        )
                # bias: += b ⊗ ones_s  (K=1)
                nc.tensor.matmul(
                    ps[:], lhsT=b_sb[:], rhs=ones_s[:],
                    start=False, stop=True,
                )
                nc.vector.tensor_copy(dst_ap, ps[:])
                return ps

            def attn(qt, h):
                nkb = 4 * qt + 4
                po = psO.tile([65, ST], f32, tag="po", name=f"po{qt}_{h}")
                for pr in range(nkb // 2):
                    ps = psS.tile([128, 2 * ST], f32, tag="ps",
                                  name=f"ps{qt}_{h}_{pr}")
                    for j in range(2):
                        kb = 2 * pr + j
                        nc.tensor.matmul(
                            ps[:, j * ST : (j + 1) * ST],
                            lhsT=(kT_sb[64 * h : 64 * h + 64,
                                         kb * KB : (kb + 1) * KB]),
                            rhs=(qT_sb[64 * h : 64 * h + 64,
                                        qt * ST : (qt + 1) * ST]),
                            start=True,
                            stop=True,
                        )
                        rel = kb - 4 * qt
                        if rel >= 0:  # diagonal block: additive causal mask
                            nc.vector.tensor_tensor(
                                out=ps[:, j * ST : (j + 1) * ST],
                                in0=ps[:, j * ST : (j + 1) * ST],
                                in1=mask_sb[:, rel * ST : (rel + 1) * ST],
                                op=ADD,
                            )
                    ex = exp_p.tile([128, 2 * ST], f32r, tag="ex",
                                    name=f"ex{qt}_{h}_{pr}")
                    nc.scalar.activation(ex[:], ps[:], EXP, scale=0.125)
                    for j in range(2):
                        kb = 2 * pr + j
                        nc.tensor.matmul(
                            po[:],
                            lhsT=(v_sb[:, kb, h * 65 : h * 65 + 65]),
                            rhs=(ex[:, j * ST : (j + 1) * ST]),
                            start=(pr == 0 and j == 0),
                            stop=(pr == nkb // 2 - 1 and j == 1),
                        )
                o_sb = osb_p.tile([65, ST], f32r, tag="o", name=f"o{qt}_{h}")
                nc.vector.tensor_copy(o_sb[:], po[:])
                # denom (row 64) broadcast to 64 partitions via K=1 matmul
                pb = psO.tile([DK, ST], f32, tag="po", name=f"pb{qt}_{h}")
                nc.tensor.matmul(
                    pb[:], lhsT=onesP[64:65, :], rhs=o_sb[64:65, :],
                    start=True, stop=True,
                )
                r_sb = rsb_p.tile([DK, ST], f32, tag="r", name=f"r{qt}_{h}")
                nc.vector.reciprocal(r_sb[:], pb[:])
                woL = woL0 if h == 0 else woL1
                nc.vector.tensor_tensor(
                    out=woL[:, qt * ST : (qt + 1) * ST],
                    in0=o_sb[0:64, :],
                    in1=r_sb[:],
                    op=MULT,
                )

            def wo(qt):
                for qb in range(4):
                    q0 = qt * ST + qb * 128
                    for nt in range(2):
                        pw = psA.tile([128, ST], f32, tag="pp",
                                      name=f"pw{qt}_{qb}_{nt}")
                        nc.tensor.matmul(
                            pw[:],
                            lhsT=(woL0[:, q0 : q0 + 128]),
                            rhs=(woR0[:, nt * ST : (nt + 1) * ST]),
                            start=True, stop=False,
                        )
                        nc.tensor.matmul(
                            pw[:],
                            lhsT=(woL1[:, q0 : q0 + 128]),
                            rhs=(woR1[:, nt * ST : (nt + 1) * ST]),
                            start=False, stop=True,
                        )
                        ob = oout_p.tile([128, ST], f32, tag="ob",
                                         name=f"ob{qt}_{qb}_{nt}")
                        nc.vector.tensor_copy(ob[:], pw[:])
                        nc.sync.dma_start(
                            out=OUT[q0 : q0 + 128, nt * ST : (nt + 1) * ST],
                            in_=ob[:],
                        )

            for st in range(NST):
                project(st, QTr, wq_sb, bq_sb, qin_p,
                        qT_sb[:, st * ST : (st + 1) * ST])
                project(st, KTr, wk_sb, bk_sb, kin_p,
                        kT_sb[:, st * ST : (st + 1) * ST])
                vtx = vtx_p.tile([128, ST], f32, tag="vtx", name=f"vtx{st}")
                project(st, VTr, wv_sb, bv_sb, vin_p, vtx[:])
                # transpose vT [c, s] blocks into v_sb [s, c] aug slots
                for qb in range(4):
                    kb = 4 * st + qb
                    tp = psA.tile([128, 128], f32, tag="pp", name=f"pt{kb}")
                    nc.tensor.transpose(
                        tp[:], vtx[:, qb * 128 : (qb + 1) * 128], eye_sb[:]
                    )
                    nc.vector.tensor_copy(v_sb[:, kb, 0:DK], tp[:, 0:DK])
                    nc.vector.tensor_copy(
                        v_sb[:, kb, DK + 1 : SLOT - 1], tp[:, DK:CD]
                    )
                attn(st, 0)
                attn(st, 1)
                if st > 0:
                    wo(st - 1)
            wo(NST - 1)

    nc.compile()
    return nc


def _prep_inputs(Q, K, V, Wq_w, Wq_b, Wk_w, Wk_b, Wv_w, Wv_b, Wo_w, Wo_b):
    f = np.float32
    QT = np.ascontiguousarray(Q[0].T, dtype=f)
    KT = np.ascontiguousarray(K[0].T, dtype=f)
    VT = np.ascontiguousarray(V[0].T, dtype=f)
    # diagonal-block additive causal masks: m[p, r*ST + f] = 0 if 128r+p<=f
    p = np.arange(KB)[:, None]
    fidx = np.arange(ST)[None, :]
    msk = np.concatenate(
        [np.where(128 * rel + p <= fidx, 0.0, -1e9) for rel in range(4)],
        axis=1,
    ).astype(f)
    eye = np.eye(128, dtype=f)
    WoT = np.ascontiguousarray(Wo_w.T, dtype=f)  # [in, out]

    in_maps = []
    for c in range(NCORES):
        c0 = CD * c
        in_maps.append({
            "qt": QT, "kt": KT, "vt": VT,
            "wq": np.ascontiguousarray(Wq_w[c0 : c0 + CD, :].T, dtype=f),
            "wk": np.ascontiguousarray(Wk_w[c0 : c0 + CD, :].T, dtype=f),
            "wv": np.ascontiguousarray(Wv_w[c0 : c0 + CD, :].T, dtype=f),
            "bq": np.ascontiguousarray(Wq_b[None, c0 : c0 + CD], dtype=f),
            "bk": np.ascontiguousarray(Wk_b[None, c0 : c0 + CD], dtype=f),
            "bv": np.ascontiguousarray(Wv_b[None, c0 : c0 + CD], dtype=f),
            "wo0": np.ascontiguousarray(WoT[c0 : c0 + DK, :], dtype=f),
            "wo1": np.ascontiguousarray(WoT[c0 + DK : c0 + CD, :], dtype=f),
            "msk": msk, "eye": eye,
            "onev": np.ones((KB, NKB, 1), f), "ones": np.ones((1, ST), f),
            "onep": np.ones((1, DK), f),
        })
    return in_maps


def _numpy_fallback(Q, K, V, Wq_w, Wq_b, Wk_w, Wk_b, Wv_w, Wv_b, Wo_w, Wo_b,
                    mask):
    q = (Q @ Wq_w.T + Wq_b).reshape(1, S, H, DK).transpose(0, 2, 1, 3)
    k = (K @ Wk_w.T + Wk_b).reshape(1, S, H, DK).transpose(0, 2, 1, 3)
    v = (V @ Wv_w.T + Wv_b).reshape(1, S, H, DK).transpose(0, 2, 1, 3)
    scores = np.einsum("bhqd,bhkd->bhqk", q, k) / np.sqrt(DK).astype(np.float32)
    scores = np.where(mask == 0, np.float32(-1e9), scores)
    scores -= scores.max(axis=-1, keepdims=True)
    e = np.exp(scores)
    attn = e / e.sum(axis=-1, keepdims=True)
    out = np.einsum("bhqk,bhkd->bhqd", attn, v)
    out = out.transpose(0, 2, 1, 3).reshape(1, S, D)
    return (out @ Wo_w.T + Wo_b).astype(np.float32)


def kernel(Q, K, V, Wq_w, Wq_b, Wk_w, Wk_b, Wv_w, Wv_b, Wo_w, Wo_b, mask,
           **run_kwargs):
    Q = np.asarray(Q); K = np.asarray(K); V = np.asarray(V)
    Wq_w = np.asarray(Wq_w); Wq_b = np.asarray(Wq_b)
    Wk_w = np.asarray(Wk_w); Wk_b = np.asarray(Wk_b)
    Wv_w = np.asarray(Wv_w); Wv_b = np.asarray(Wv_b)
    Wo_w = np.asarray(Wo_w); Wo_b = np.asarray(Wo_b)
    mask = np.asarray(mask)

    causal = np.array_equal(
        mask.reshape(S, S), np.tril(np.ones((S, S), mask.dtype))
    )
    if not causal:
        return _numpy_fallback(Q, K, V, Wq_w, Wq_b, Wk_w, Wk_b, Wv_w, Wv_b,
                               Wo_w, Wo_b, mask)

    from concourse.bass_utils import run_bass_kernel_spmd

    if _compiled[0] is None:
        _compiled[0] = _build()
    nc = _compiled[0]

    in_maps = _prep_inputs(Q, K, V, Wq_w, Wq_b, Wk_w, Wk_b, Wv_w, Wv_b,
                           Wo_w, Wo_b)
    res = run_bass_kernel_spmd(nc, in_maps, list(range(NCORES)), **run_kwargs)
    out = np.zeros((S, D), np.float32)
    for cres in res.results:
        out += cres["out"]
    out += Wo_b.astype(np.float32)
    if run_kwargs:
        kernel.last_result = res
    return out.reshape(1, S, D)
